# revision 12
# baseline (speedup 1.0000x reference)
"""Trainium2 Bass kernel for nn_MultiHeadAttention_55336358642102.

Strategy: data-parallel over the 8 equal-length sentences (B=8) — one
sentence per NeuronCore, no collectives. Each core computes, for its
[L=1024, D=1024] slice:
  - Q^T/K^T/V^T per head via weight-stationary matmuls (heads packed in
    pairs so the PE runs with M=128), operating on a host-pretransposed
    X^T so the contraction dim (features) sits on partitions.
  - attention in "transposed score" space: S^T = K^T-chunks.T @ Q^T so the
    softmaxed probabilities come out with keys on partitions, which is the
    exact layout the P@V matmul needs (lhsT = V-natural chunks).
  - softmax without max-subtraction (logits are ~N(0, 0.15) here — exact
    softmax is shift-invariant so this matches the reference); the
    denominator comes from an extra ones-vector matmul over exp(S^T).
  - output projection with the per-head halves packed into two [512, L]
    operands (O1T/O2T) whose row order matches a host-permuted
    w_proj1/w_proj2, then residual + unbiased-std layernorm in fp32.

Matmul operands are bf16 (full PE rate); accumulation, residual and
layernorm are fp32.
"""

import sys

import ml_dtypes
import numpy as np

if "/opt/trn_rl_repo" not in sys.path:
    sys.path.insert(0, "/opt/trn_rl_repo")

import concourse.bass as bass
import concourse.mybir as mybir
import concourse.tile as tile
from concourse import bacc
from concourse.bass import ds
from concourse.bass_utils import run_bass_kernel_spmd
from concourse.masks import make_identity

P = 128
L = 1024            # rows per core (= max_len; one sentence per core)
DM = 1024           # d_model
DC, DP = 768, 256   # content / positional feature split
NKC, NKP = DC // P, DP // P     # 6, 2 feature chunks
NPAIR = 4
NCORES = 8
INV_TEMPER = 1.0 / 32.0         # 1/sqrt(DM)
EPS = 1e-3
F32 = mybir.dt.float32
BF16 = mybir.dt.bfloat16
AF = mybir.ActivationFunctionType
ALU = mybir.AluOpType
BF16NP = ml_dtypes.bfloat16


def build_nc(apply_ln: bool) -> bass.Bass:
    nc = bacc.Bacc(None, target_bir_lowering=False)

    xt = nc.dram_tensor("xt", [DM, L], BF16, kind="ExternalInput")
    xr = nc.dram_tensor("xr", [L, DM], F32, kind="ExternalInput")
    wc_d = nc.dram_tensor("wc", [NPAIR, DC, 3, P], BF16, kind="ExternalInput")
    wp_d = nc.dram_tensor("wp", [NPAIR, DP, 3, P], BF16, kind="ExternalInput")
    w1_d = nc.dram_tensor("w1", [512, DC], BF16, kind="ExternalInput")
    w2_d = nc.dram_tensor("w2", [512, DP], BF16, kind="ExternalInput")
    if apply_ln:
        lna_d = nc.dram_tensor("lna", [1, DM], F32, kind="ExternalInput")
        lnb_d = nc.dram_tensor("lnb", [1, DM], F32, kind="ExternalInput")
    out_d = nc.dram_tensor("out", [L, DM], F32, kind="ExternalOutput")

    with tile.TileContext(nc) as tc:
        with (
            tc.tile_pool(name="sing", bufs=1) as sing,
            tc.tile_pool(name="wpool", bufs=2) as wpool,
            tc.tile_pool(name="qkt", bufs=2) as qkt,
            tc.tile_pool(name="epool", bufs=3) as epool,
            tc.tile_pool(name="dpool", bufs=4) as dpool,
            tc.tile_pool(name="zpool", bufs=2) as zpool,
            tc.tile_pool(name="xpool", bufs=2) as xpool,
            tc.tile_pool(name="stat", bufs=3) as stat,
            tc.tile_pool(name="dram", bufs=4, space="DRAM") as drampool,
            tc.tile_pool(name="ps_mm", bufs=3, space="PSUM") as ps_mm,
            tc.tile_pool(name="ps_pv", bufs=3, space="PSUM") as ps_pv,
            tc.tile_pool(name="ps_d", bufs=2, space="PSUM") as ps_d,
        ):
            # ---- resident constants -------------------------------------
            XT = sing.tile([P, DM // P, L], BF16)   # X^T: [feat%128, chunk, t]
            nc.sync.dma_start(XT, xt.rearrange("(o p) t -> p o t", p=P))

            ident = sing.tile([P, P], BF16)
            make_identity(nc, ident)
            ones = sing.tile([P, 1], BF16)
            nc.vector.memset(ones, 1.0)

            W1 = sing.tile([P, 4, DC], BF16)
            nc.sync.dma_start(W1, w1_d.rearrange("(o p) n -> p o n", p=P))
            W2 = sing.tile([P, 4, DP], BF16)
            nc.sync.dma_start(W2, w2_d.rearrange("(o p) n -> p o n", p=P))
            if apply_ln:
                LNA = sing.tile([1, DM], F32)
                nc.sync.dma_start(LNA, lna_d)
                LNB = sing.tile([1, DM], F32)
                nc.sync.dma_start(LNB, lnb_d)

            O1T = sing.tile([P, 4, L], BF16)   # packed (head, dv<64) rows x t
            O2T = sing.tile([P, 4, L], BF16)

            lo = slice(0, 64)
            hi = slice(64, 128)

            for j in range(NPAIR):
                # ---- Phase A: QKV for head pair (2j, 2j+1) --------------
                wc = wpool.tile([P, NKC, 3, P], BF16, tag="wc")
                nc.sync.dma_start(wc, wc_d[j].rearrange("(o p) s m -> p o s m", p=P))
                wp = wpool.tile([P, NKP, 3, P], BF16, tag="wp")
                nc.sync.dma_start(wp, wp_d[j].rearrange("(o p) s m -> p o s m", p=P))

                QT = qkt.tile([P, 2, L], BF16, tag="qt")
                KT = qkt.tile([P, 2, L], BF16, tag="kt")
                VT = qkt.tile([P, 2, L], BF16, tag="vt")
                V = qkt.tile([P, 2, 8, P], BF16, tag="v")

                for s, DST in ((0, QT), (1, KT), (2, VT)):
                    for half in range(2):
                        hs = ds(half * 512, 512)
                        pc = ps_mm.tile([P, 512], F32, tag="mm")
                        for kc in range(NKC):
                            nc.tensor.matmul(
                                pc, wc[:, kc, s, :], XT[:, kc, hs],
                                start=(kc == 0), stop=(kc == NKC - 1))
                        pp = ps_mm.tile([P, 512], F32, tag="mm")
                        for kc in range(NKP):
                            nc.tensor.matmul(
                                pp, wp[:, kc, s, :], XT[:, NKC + kc, hs],
                                start=(kc == 0), stop=(kc == NKP - 1))
                        # psum -> sbuf staging (partition-preserving copies,
                        # cast to bf16)
                        nc.any.tensor_copy(DST[lo, 0, hs], pc[lo])
                        nc.any.tensor_copy(DST[hi, 1, hs], pc[hi])
                        nc.any.tensor_copy(DST[lo, 1, hs], pp[lo])
                        nc.any.tensor_copy(DST[hi, 0, hs], pp[hi])

                # V natural layout via PE transposes of V^T chunks
                for hh in range(2):
                    for c in range(8):
                        ptf = ps_mm.tile([P, 512], BF16, tag="mm",
                                         name=f"tp_{j}_{hh}_{c}")
                        pt = ptf[:, 0:P]
                        nc.tensor.transpose(
                            pt, VT[:, hh, ds(c * P, P)], ident)
                        nc.any.tensor_copy(V[:, hh, c, :], pt)

                # ---- Phase B: attention for the two heads ---------------
                for hh in range(2):
                    pv0 = ps_pv.tile([P, 512], F32, tag="pv")
                    pv1 = ps_pv.tile([P, 512], F32, tag="pv")
                    dd0 = ps_d.tile([1, 512], F32, tag="d")
                    dd1 = ps_d.tile([1, 512], F32, tag="d")
                    for c in range(8):
                        for half, pv, dd in ((0, pv0, dd0), (1, pv1, dd1)):
                            hs = ds(half * 512, 512)
                            pss = ps_mm.tile([P, 512], F32, tag="mm")
                            nc.tensor.matmul(
                                pss, KT[:, hh, ds(c * P, P)],
                                QT[:, hh, hs], start=True, stop=True)
                            e = epool.tile([P, 512], BF16, tag="e")
                            nc.scalar.activation(e, pss, AF.Exp, scale=INV_TEMPER)
                            nc.tensor.matmul(
                                pv, V[:, hh, c, :], e,
                                start=(c == 0), stop=(c == 7))
                            nc.tensor.matmul(
                                dd, ones, e,
                                start=(c == 0), stop=(c == 7))

                    # normalize: out^T[dv, q] * (1/d[q]); 1/d computed on DVE
                    # then replicated across partitions via a DRAM bounce
                    for half, pv, dd in ((0, pv0, dd0), (1, pv1, dd1)):
                        ds1 = dpool.tile([1, 512], F32, tag="ds1")
                        nc.vector.reciprocal(ds1, dd)   # psum -> sbuf, fused
                        dsd = drampool.tile([1, 512], F32, tag="dsd")
                        nc.sync.dma_start(dsd, ds1)
                        rd = dpool.tile([P, 512], F32, tag="rd")
                        nc.sync.dma_start(rd, dsd.to_broadcast((P, 512)))
                        hs = ds(half * 512, 512)
                        if hh == 0:   # psum rows: [o1 | o2]
                            nc.vector.tensor_mul(O1T[lo, j, hs], pv[lo], rd[lo])
                            nc.vector.tensor_mul(O2T[hi, j, hs], pv[hi], rd[hi])
                        else:         # psum rows: [o2 | o1]
                            nc.vector.tensor_mul(O2T[lo, j, hs], pv[lo], rd[lo])
                            nc.vector.tensor_mul(O1T[hi, j, hs], pv[hi], rd[hi])

            # ---- Phase C: output projection + residual + layernorm ------
            for t in range(L // P):
                tsl = ds(t * P, P)
                poa = ps_pv.tile([P, 512], F32, tag="pv")   # o1[:, 0:512]
                pob = ps_pv.tile([P, 512], F32, tag="pv")   # o1[:,512:768] | o2
                for kc in range(4):
                    nc.tensor.matmul(poa, O1T[:, kc, tsl],
                                     W1[:, kc, 0:512],
                                     start=kc == 0, stop=kc == 3)
                for kc in range(4):
                    nc.tensor.matmul(pob[:, 0:256], O1T[:, kc, tsl],
                                     W1[:, kc, 512:768],
                                     start=kc == 0, stop=kc == 3)
                for kc in range(4):
                    nc.tensor.matmul(pob[:, 256:512], O2T[:, kc, tsl],
                                     W2[:, kc, :],
                                     start=kc == 0, stop=kc == 3)

                xts = xpool.tile([P, DM], F32, tag="x")
                nc.sync.dma_start(xts, xr[tsl, :])
                z = zpool.tile([P, DM], F32, tag="z")
                nc.vector.tensor_add(z[:, 0:512], poa, xts[:, 0:512])
                nc.vector.tensor_add(z[:, 512:1024], pob, xts[:, 512:1024])

                stats = stat.tile([P, 2, 6], F32, tag="st")
                nc.vector.bn_stats(stats[:, 0], z[:, 0:512])
                nc.vector.bn_stats(stats[:, 1], z[:, 512:1024])
                mv = stat.tile([P, 2], F32, tag="mv")
                nc.vector.bn_aggr(mv, stats)
                sig = stat.tile([P, 1], F32, tag="sig")
                # unbiased std: sqrt(var * n/(n-1)), then +eps, then 1/x
                nc.scalar.activation(sig, mv[:, 1:2], AF.Sqrt,
                                     scale=float(DM) / (DM - 1))
                nc.vector.tensor_scalar_add(sig, sig, EPS)
                nc.vector.reciprocal(sig, sig)
                nc.vector.tensor_scalar(z, z, mv[:, 0:1], sig,
                                        ALU.subtract, ALU.mult)
                if apply_ln:
                    nc.vector.tensor_mul(z, z, LNA.to_broadcast((P, DM)))
                    nc.vector.tensor_add(z, z, LNB.to_broadcast((P, DM)))
                nc.sync.dma_start(out_d[tsl, :], z)

    nc.finalize()
    return nc


def _prep(inp, w_qs1, w_ks1, w_vs1, w_qs2, w_ks2, w_vs2, w_proj1, w_proj2):
    wc = np.empty((NPAIR, DC, 3, P), BF16NP)
    wp = np.empty((NPAIR, DP, 3, P), BF16NP)
    for j in range(NPAIR):
        for s, (wa, wb) in enumerate(((w_qs1, w_qs2), (w_ks1, w_ks2),
                                      (w_vs1, w_vs2))):
            wc[j, :, s, 0:64] = wa[2 * j]
            wc[j, :, s, 64:128] = wa[2 * j + 1]
            wp[j, :, s, 0:64] = wb[2 * j + 1]   # pos halves swapped
            wp[j, :, s, 64:128] = wb[2 * j]
    w1 = np.asarray(w_proj1, np.float32).astype(BF16NP)
    w2f = np.asarray(w_proj2, np.float32)
    w2 = np.empty((512, DP), BF16NP)
    for j in range(NPAIR):
        w2[j * 128: j * 128 + 64] = w2f[(2 * j + 1) * 64: (2 * j + 2) * 64]
        w2[j * 128 + 64: (j + 1) * 128] = w2f[(2 * j) * 64: (2 * j + 1) * 64]

    x = np.ascontiguousarray(np.asarray(inp, np.float32)).reshape(NCORES, L, DM)
    xts = [np.ascontiguousarray(x[b].T.astype(BF16NP)) for b in range(NCORES)]
    return x, xts, wc, wp, w1, w2


_NC_CACHE = {}


def _get_nc(apply_ln):
    if apply_ln not in _NC_CACHE:
        _NC_CACHE[apply_ln] = build_nc(apply_ln)
    return _NC_CACHE[apply_ln]


def kernel(inp, w_qs1, w_ks1, w_vs1, w_qs2, w_ks2, w_vs2, w_proj1, w_proj2,
           ln_a, ln_b, batch_size, max_len, _trace=False):
    inp = np.asarray(inp, np.float32)
    assert int(batch_size) == NCORES and int(max_len) == L
    assert inp.shape == (NCORES * L, DM)

    ln_a = np.asarray(ln_a, np.float32).reshape(-1)
    ln_b = np.asarray(ln_b, np.float32).reshape(-1)
    apply_ln = not (np.all(ln_a == 1.0) and np.all(ln_b == 0.0))

    x, xts, wc, wp, w1, w2 = _prep(
        inp, np.asarray(w_qs1, np.float32), np.asarray(w_ks1, np.float32),
        np.asarray(w_vs1, np.float32), np.asarray(w_qs2, np.float32),
        np.asarray(w_ks2, np.float32), np.asarray(w_vs2, np.float32),
        np.asarray(w_proj1, np.float32), np.asarray(w_proj2, np.float32))

    nc = _get_nc(apply_ln)

    in_maps = []
    for b in range(NCORES):
        m = dict(xt=xts[b], xr=np.ascontiguousarray(x[b]),
                 wc=wc, wp=wp, w1=w1, w2=w2)
        if apply_ln:
            m["lna"] = ln_a.reshape(1, DM)
            m["lnb"] = ln_b.reshape(1, DM)
        in_maps.append(m)

    res = run_bass_kernel_spmd(nc, in_maps, list(range(NCORES)), trace=_trace)
    out = np.concatenate([res.results[b]["out"] for b in range(NCORES)], 0)
    if _trace:
        return out, res
    return out


# revision 14
# speedup vs baseline: 1.0859x; 1.0859x over previous
"""Trainium2 Bass kernel for nn_MultiHeadAttention_55336358642102.

Strategy: data-parallel over the 8 equal-length sentences (B=8) — one
sentence per NeuronCore, no collectives. Each core computes, for its
[L=1024, D=1024] slice:
  - Q^T/K^T/V^T per head via weight-stationary matmuls (heads packed in
    pairs so the PE runs with M=128), operating on a host-pretransposed
    X^T so the contraction dim (features) sits on partitions.
  - attention in "transposed score" space: S^T = K^T-chunks.T @ Q^T so the
    softmaxed probabilities come out with keys on partitions, which is the
    exact layout the P@V matmul needs (lhsT = V-natural chunks).
  - softmax without max-subtraction (logits are ~N(0, 0.15) here — exact
    softmax is shift-invariant so this matches the reference); the
    denominator comes from an extra ones-vector matmul over exp(S^T).
  - output projection with the per-head halves packed into two [512, L]
    operands (O1T/O2T) whose row order matches a host-permuted
    w_proj1/w_proj2, then residual + unbiased-std layernorm in fp32.

Matmul operands are bf16 (full PE rate); accumulation, residual and
layernorm are fp32.
"""

import sys

import ml_dtypes
import numpy as np

if "/opt/trn_rl_repo" not in sys.path:
    sys.path.insert(0, "/opt/trn_rl_repo")

import concourse.bass as bass
import concourse.mybir as mybir
import concourse.tile as tile
from concourse import bacc
from concourse.bass import ds
from concourse.bass_utils import run_bass_kernel_spmd
from concourse.masks import make_identity

P = 128
L = 1024            # rows per core (= max_len; one sentence per core)
DM = 1024           # d_model
DC, DP = 768, 256   # content / positional feature split
NKC, NKP = DC // P, DP // P     # 6, 2 feature chunks
NPAIR = 4
NCORES = 8
INV_TEMPER = 1.0 / 32.0         # 1/sqrt(DM)
EPS = 1e-3
F32 = mybir.dt.float32
BF16 = mybir.dt.bfloat16
AF = mybir.ActivationFunctionType
ALU = mybir.AluOpType
BF16NP = ml_dtypes.bfloat16


def build_nc(apply_ln: bool) -> bass.Bass:
    nc = bacc.Bacc(None, target_bir_lowering=False)

    # all inputs are pre-arranged on the host to be partition-major and
    # contiguous per partition, so every load is a single 2D DMA pattern
    xt = nc.dram_tensor("xt", [P, DM // P, L], BF16, kind="ExternalInput")
    xr = nc.dram_tensor("xr", [L, DM], F32, kind="ExternalInput")
    wc_d = nc.dram_tensor("wc", [P, NPAIR, NKC, 3, P], BF16, kind="ExternalInput")
    wp_d = nc.dram_tensor("wp", [P, NPAIR, NKP, 3, P], BF16, kind="ExternalInput")
    w1_d = nc.dram_tensor("w1", [P, 4, DC], BF16, kind="ExternalInput")
    w2_d = nc.dram_tensor("w2", [P, 4, DP], BF16, kind="ExternalInput")
    if apply_ln:
        lna_d = nc.dram_tensor("lna", [1, DM], F32, kind="ExternalInput")
        lnb_d = nc.dram_tensor("lnb", [1, DM], F32, kind="ExternalInput")
    out_d = nc.dram_tensor("out", [L, DM], F32, kind="ExternalOutput")

    with tile.TileContext(nc) as tc:
        with (
            tc.tile_pool(name="sing", bufs=1) as sing,
            tc.tile_pool(name="wpool", bufs=2) as wpool,
            tc.tile_pool(name="qkt", bufs=2) as qkt,
            tc.tile_pool(name="epool", bufs=3) as epool,
            tc.tile_pool(name="dpool", bufs=4) as dpool,
            tc.tile_pool(name="zpool", bufs=2) as zpool,
            tc.tile_pool(name="xpool", bufs=2) as xpool,
            tc.tile_pool(name="stat", bufs=3) as stat,
            tc.tile_pool(name="dram", bufs=4, space="DRAM") as drampool,
            tc.tile_pool(name="ps_mm", bufs=3, space="PSUM") as ps_mm,
            tc.tile_pool(name="ps_pv", bufs=3, space="PSUM") as ps_pv,
            tc.tile_pool(name="ps_d", bufs=2, space="PSUM") as ps_d,
        ):
            # ---- resident constants -------------------------------------
            XT = sing.tile([P, DM // P, L], BF16)   # X^T: [feat%128, chunk, t]
            nc.sync.dma_start(XT, xt[:])

            ident = sing.tile([P, P], BF16)
            make_identity(nc, ident)
            ones = sing.tile([P, P], BF16)
            nc.vector.memset(ones, 1.0)

            W1 = sing.tile([P, 4, DC], BF16)
            nc.sync.dma_start(W1, w1_d[:])
            W2 = sing.tile([P, 4, DP], BF16)
            nc.sync.dma_start(W2, w2_d[:])
            if apply_ln:
                LNA = sing.tile([1, DM], F32)
                nc.sync.dma_start(LNA, lna_d)
                LNB = sing.tile([1, DM], F32)
                nc.sync.dma_start(LNB, lnb_d)

            O1T = sing.tile([P, 4, L], BF16)   # packed (head, dv<64) rows x t
            O2T = sing.tile([P, 4, L], BF16)

            lo = slice(0, 64)
            hi = slice(64, 128)

            for j in range(NPAIR):
                # ---- Phase A: QKV for head pair (2j, 2j+1) --------------
                wc = wpool.tile([P, NKC, 3, P], BF16, tag="wc")
                nc.sync.dma_start(wc, wc_d[:, j])
                wp = wpool.tile([P, NKP, 3, P], BF16, tag="wp")
                nc.sync.dma_start(wp, wp_d[:, j])

                QT = qkt.tile([P, 2, L], BF16, tag="qt")
                KT = qkt.tile([P, 2, L], BF16, tag="kt")
                VT = qkt.tile([P, 2, L], BF16, tag="vt")
                V = qkt.tile([P, 2, 8, P], BF16, tag="v")

                for s, DST in ((0, QT), (1, KT), (2, VT)):
                    for half in range(2):
                        hs = ds(half * 512, 512)
                        pc = ps_mm.tile([P, 512], F32, tag="mm")
                        for kc in range(NKC):
                            nc.tensor.matmul(
                                pc, wc[:, kc, s, :], XT[:, kc, hs],
                                start=(kc == 0), stop=(kc == NKC - 1))
                        pp = ps_mm.tile([P, 512], F32, tag="mm")
                        for kc in range(NKP):
                            nc.tensor.matmul(
                                pp, wp[:, kc, s, :], XT[:, NKC + kc, hs],
                                start=(kc == 0), stop=(kc == NKP - 1))
                        # psum -> sbuf staging (partition-preserving copies,
                        # cast to bf16)
                        nc.any.tensor_copy(DST[lo, 0, hs], pc[lo])
                        nc.any.tensor_copy(DST[hi, 1, hs], pc[hi])
                        nc.any.tensor_copy(DST[lo, 1, hs], pp[lo])
                        nc.any.tensor_copy(DST[hi, 0, hs], pp[hi])

                # V natural layout via PE transposes of V^T chunks
                for hh in range(2):
                    for c in range(8):
                        ptf = ps_mm.tile([P, 512], BF16, tag="mm",
                                         name=f"tp_{j}_{hh}_{c}")
                        pt = ptf[:, 0:P]
                        nc.tensor.transpose(
                            pt, VT[:, hh, ds(c * P, P)], ident)
                        nc.any.tensor_copy(V[:, hh, c, :], pt)

                # ---- Phase B: attention for the two heads ---------------
                for hh in range(2):
                    pv0 = ps_pv.tile([P, 512], F32, tag="pv")
                    pv1 = ps_pv.tile([P, 512], F32, tag="pv")
                    dd0 = ps_d.tile([P, 512], F32, tag="d")
                    dd1 = ps_d.tile([P, 512], F32, tag="d")
                    for c in range(8):
                        for half, pv, dd in ((0, pv0, dd0), (1, pv1, dd1)):
                            hs = ds(half * 512, 512)
                            pss = ps_mm.tile([P, 512], F32, tag="mm")
                            nc.tensor.matmul(
                                pss, KT[:, hh, ds(c * P, P)],
                                QT[:, hh, hs], start=True, stop=True)
                            e = epool.tile([P, 512], BF16, tag="e")
                            nc.scalar.activation(e, pss, AF.Exp, scale=INV_TEMPER)
                            nc.tensor.matmul(
                                pv, V[:, hh, c, :], e,
                                start=(c == 0), stop=(c == 7))
                            nc.tensor.matmul(
                                dd, ones, e,
                                start=(c == 0), stop=(c == 7))

                    # normalize: out^T[dv, q] * (1/d[q]); the all-ones lhsT
                    # already replicated d across every psum partition
                    for half, pv, dd in ((0, pv0, dd0), (1, pv1, dd1)):
                        rd = dpool.tile([P, 512], F32, tag="rd")
                        nc.vector.reciprocal(rd, dd)   # psum -> sbuf, fused
                        hs = ds(half * 512, 512)
                        if hh == 0:   # psum rows: [o1 | o2]
                            nc.vector.tensor_mul(O1T[lo, j, hs], pv[lo], rd[lo])
                            nc.vector.tensor_mul(O2T[hi, j, hs], pv[hi], rd[hi])
                        else:         # psum rows: [o2 | o1]
                            nc.vector.tensor_mul(O2T[lo, j, hs], pv[lo], rd[lo])
                            nc.vector.tensor_mul(O1T[hi, j, hs], pv[hi], rd[hi])

            # ---- Phase C: output projection + residual + layernorm ------
            for t in range(L // P):
                tsl = ds(t * P, P)
                poa = ps_pv.tile([P, 512], F32, tag="pv")   # o1[:, 0:512]
                pob = ps_pv.tile([P, 512], F32, tag="pv")   # o1[:,512:768] | o2
                for kc in range(4):
                    nc.tensor.matmul(poa, O1T[:, kc, tsl],
                                     W1[:, kc, 0:512],
                                     start=kc == 0, stop=kc == 3)
                for kc in range(4):
                    nc.tensor.matmul(pob[:, 0:256], O1T[:, kc, tsl],
                                     W1[:, kc, 512:768],
                                     start=kc == 0, stop=kc == 3)
                for kc in range(4):
                    nc.tensor.matmul(pob[:, 256:512], O2T[:, kc, tsl],
                                     W2[:, kc, :],
                                     start=kc == 0, stop=kc == 3)

                xts = xpool.tile([P, DM], F32, tag="x")
                nc.sync.dma_start(xts, xr[tsl, :])
                z = zpool.tile([P, DM], F32, tag="z")
                nc.vector.tensor_add(z[:, 0:512], poa, xts[:, 0:512])
                nc.vector.tensor_add(z[:, 512:1024], pob, xts[:, 512:1024])

                stats = stat.tile([P, 2, 6], F32, tag="st")
                nc.vector.bn_stats(stats[:, 0], z[:, 0:512])
                nc.vector.bn_stats(stats[:, 1], z[:, 512:1024])
                mv = stat.tile([P, 2], F32, tag="mv")
                nc.vector.bn_aggr(mv, stats)
                sig = stat.tile([P, 1], F32, tag="sig")
                # unbiased std: sqrt(var * n/(n-1)), then +eps, then 1/x
                nc.scalar.activation(sig, mv[:, 1:2], AF.Sqrt,
                                     scale=float(DM) / (DM - 1))
                nc.vector.tensor_scalar_add(sig, sig, EPS)
                nc.vector.reciprocal(sig, sig)
                nc.vector.tensor_scalar(z, z, mv[:, 0:1], sig,
                                        ALU.subtract, ALU.mult)
                if apply_ln:
                    nc.vector.tensor_mul(z, z, LNA.to_broadcast((P, DM)))
                    nc.vector.tensor_add(z, z, LNB.to_broadcast((P, DM)))
                nc.sync.dma_start(out_d[tsl, :], z)

    nc.finalize()
    return nc


def _part_major(a, p=P):
    """[K*p, ...rest] -> [p, K, ...rest] contiguous (partition-major)."""
    k = a.shape[0] // p
    return np.ascontiguousarray(
        a.reshape((k, p) + a.shape[1:]).swapaxes(0, 1))


def _prep(inp, w_qs1, w_ks1, w_vs1, w_qs2, w_ks2, w_vs2, w_proj1, w_proj2):
    wc = np.empty((NPAIR, DC, 3, P), BF16NP)
    wp = np.empty((NPAIR, DP, 3, P), BF16NP)
    for j in range(NPAIR):
        for s, (wa, wb) in enumerate(((w_qs1, w_qs2), (w_ks1, w_ks2),
                                      (w_vs1, w_vs2))):
            wc[j, :, s, 0:64] = wa[2 * j]
            wc[j, :, s, 64:128] = wa[2 * j + 1]
            wp[j, :, s, 0:64] = wb[2 * j + 1]   # pos halves swapped
            wp[j, :, s, 64:128] = wb[2 * j]
    # -> [P, NPAIR, NK, 3, P] partition-major
    wc = np.ascontiguousarray(
        wc.reshape(NPAIR, NKC, P, 3, P).transpose(2, 0, 1, 3, 4))
    wp = np.ascontiguousarray(
        wp.reshape(NPAIR, NKP, P, 3, P).transpose(2, 0, 1, 3, 4))
    w1 = _part_major(np.asarray(w_proj1, np.float32).astype(BF16NP))
    w2f = np.asarray(w_proj2, np.float32)
    w2 = np.empty((512, DP), BF16NP)
    for j in range(NPAIR):
        w2[j * 128: j * 128 + 64] = w2f[(2 * j + 1) * 64: (2 * j + 2) * 64]
        w2[j * 128 + 64: (j + 1) * 128] = w2f[(2 * j) * 64: (2 * j + 1) * 64]
    w2 = _part_major(w2)

    x = np.ascontiguousarray(np.asarray(inp, np.float32)).reshape(NCORES, L, DM)
    xts = [_part_major(x[b].T.astype(BF16NP)) for b in range(NCORES)]
    return x, xts, wc, wp, w1, w2


_NC_CACHE = {}


def _get_nc(apply_ln):
    if apply_ln not in _NC_CACHE:
        _NC_CACHE[apply_ln] = build_nc(apply_ln)
    return _NC_CACHE[apply_ln]


def kernel(inp, w_qs1, w_ks1, w_vs1, w_qs2, w_ks2, w_vs2, w_proj1, w_proj2,
           ln_a, ln_b, batch_size, max_len, _trace=False):
    inp = np.asarray(inp, np.float32)
    assert int(batch_size) == NCORES and int(max_len) == L
    assert inp.shape == (NCORES * L, DM)

    ln_a = np.asarray(ln_a, np.float32).reshape(-1)
    ln_b = np.asarray(ln_b, np.float32).reshape(-1)
    apply_ln = not (np.all(ln_a == 1.0) and np.all(ln_b == 0.0))

    x, xts, wc, wp, w1, w2 = _prep(
        inp, np.asarray(w_qs1, np.float32), np.asarray(w_ks1, np.float32),
        np.asarray(w_vs1, np.float32), np.asarray(w_qs2, np.float32),
        np.asarray(w_ks2, np.float32), np.asarray(w_vs2, np.float32),
        np.asarray(w_proj1, np.float32), np.asarray(w_proj2, np.float32))

    nc = _get_nc(apply_ln)

    in_maps = []
    for b in range(NCORES):
        m = dict(xt=xts[b], xr=np.ascontiguousarray(x[b]),
                 wc=wc, wp=wp, w1=w1, w2=w2)
        if apply_ln:
            m["lna"] = ln_a.reshape(1, DM)
            m["lnb"] = ln_b.reshape(1, DM)
        in_maps.append(m)

    res = run_bass_kernel_spmd(nc, in_maps, list(range(NCORES)), trace=_trace)
    out = np.concatenate([res.results[b]["out"] for b in range(NCORES)], 0)
    if _trace:
        return out, res
    return out


# revision 17
# speedup vs baseline: 1.4349x; 1.3215x over previous
"""Trainium2 Bass kernel for nn_MultiHeadAttention_55336358642102.

Strategy: data-parallel over the 8 equal-length sentences (B=8) — one
sentence per NeuronCore, no collectives. Each core computes, for its
[L=1024, D=1024] slice:
  - Q^T/K^T per head via weight-stationary matmuls (heads packed in pairs
    so the PE runs with M=128) on a host-pretransposed X^T; V in natural
    [token, dv] layout directly (lhsT = X^T chunks).
  - attention in "transposed score" space: S^T = K^T-chunks.T @ Q^T so the
    softmaxed probabilities come out with keys on partitions, which is the
    exact layout the P@V matmul needs (lhsT = V-natural chunks).
  - softmax without max-subtraction (logits are ~N(0, 0.15) here — exact
    softmax is shift-invariant so this matches the reference); the
    denominator comes from an all-ones-lhsT matmul over exp(S^T), which
    also replicates it across psum partitions for the normalize step.
  - output projection with the per-head halves packed into two [512, L]
    operands (O1T/O2T) matching w_proj1/w_proj2 row order, then residual +
    unbiased-std layernorm in fp32.

Matmul operands are bf16 (full PE rate); accumulation, residual and
layernorm are fp32. All DRAM inputs are pre-arranged partition-major so
every load is one 2D DMA. Partition-range routing (head halves into
packed operands) is done with SBUF->SBUF DMAs, which unlike the compute
engines can shift partitions.
"""

import sys

import ml_dtypes
import numpy as np

if "/opt/trn_rl_repo" not in sys.path:
    sys.path.insert(0, "/opt/trn_rl_repo")

import concourse.bass as bass
import concourse.mybir as mybir
import concourse.tile as tile
from concourse import bacc
from concourse.bass import ds
from concourse.bass_utils import run_bass_kernel_spmd

P = 128
L = 1024            # rows per core (= max_len; one sentence per core)
DM = 1024           # d_model
DC, DP = 768, 256   # content / positional feature split
NKC, NKP = DC // P, DP // P     # 6, 2 feature chunks
NPAIR = 4
NCORES = 8
INV_TEMPER = 1.0 / 32.0         # 1/sqrt(DM)
EPS = 1e-3
F32 = mybir.dt.float32
BF16 = mybir.dt.bfloat16
AF = mybir.ActivationFunctionType
ALU = mybir.AluOpType
BF16NP = ml_dtypes.bfloat16


def build_nc(apply_ln: bool) -> bass.Bass:
    nc = bacc.Bacc(None, target_bir_lowering=False)

    # all inputs are pre-arranged on the host to be partition-major and
    # contiguous per partition, so every load is a single 2D DMA pattern
    xt = nc.dram_tensor("xt", [P, DM // P, L], BF16, kind="ExternalInput")
    xr = nc.dram_tensor("xr", [L, DM], F32, kind="ExternalInput")
    wc_d = nc.dram_tensor("wc", [P, NPAIR, NKC, 3, P], BF16, kind="ExternalInput")
    wp_d = nc.dram_tensor("wp", [P, NPAIR, NKP, 3, P], BF16, kind="ExternalInput")
    w1_d = nc.dram_tensor("w1", [P, 4, DC], BF16, kind="ExternalInput")
    w2_d = nc.dram_tensor("w2", [P, 4, DP], BF16, kind="ExternalInput")
    if apply_ln:
        lna_d = nc.dram_tensor("lna", [1, DM], F32, kind="ExternalInput")
        lnb_d = nc.dram_tensor("lnb", [1, DM], F32, kind="ExternalInput")
    out_d = nc.dram_tensor("out", [L, DM], F32, kind="ExternalOutput")

    with tile.TileContext(nc) as tc:
        with (
            tc.tile_pool(name="sing", bufs=1) as sing,
            tc.tile_pool(name="wpool", bufs=2) as wpool,
            tc.tile_pool(name="qkt", bufs=2) as qkt,
            tc.tile_pool(name="epool", bufs=4) as epool,
            tc.tile_pool(name="dpool", bufs=4) as dpool,
            tc.tile_pool(name="stg", bufs=6) as stg,
            tc.tile_pool(name="zpool", bufs=2) as zpool,
            tc.tile_pool(name="xpool", bufs=2) as xpool,
            tc.tile_pool(name="stat", bufs=3) as stat,
            tc.tile_pool(name="ps_mm", bufs=4, space="PSUM") as ps_mm,
            tc.tile_pool(name="ps_pv", bufs=2, space="PSUM") as ps_pv,
            tc.tile_pool(name="ps_d", bufs=2, space="PSUM") as ps_d,
        ):
            # ---- resident constants -------------------------------------
            XT = sing.tile([P, DM // P, L], BF16)   # X^T: [feat%128, chunk, t]
            nc.sync.dma_start(XT, xt[:])

            ones = sing.tile([P, P], BF16)
            nc.vector.memset(ones, 1.0)

            W1 = sing.tile([P, 4, DC], BF16)
            nc.sync.dma_start(W1, w1_d[:])
            W2 = sing.tile([P, 4, DP], BF16)
            nc.sync.dma_start(W2, w2_d[:])
            if apply_ln:
                LNA = sing.tile([1, DM], F32)
                nc.sync.dma_start(LNA, lna_d[:])
                LNB = sing.tile([1, DM], F32)
                nc.sync.dma_start(LNB, lnb_d[:])

            O1T = sing.tile([P, 4, L], BF16)   # packed (head, dv<64) rows x t
            O2T = sing.tile([P, 4, L], BF16)

            lo = slice(0, 64)
            hi = slice(64, 128)

            for j in range(NPAIR):
                # ---- Phase A: QKV for head pair (2j, 2j+1) --------------
                wc = wpool.tile([P, NKC, 3, P], BF16, tag="wc")
                nc.sync.dma_start(wc, wc_d[:, j])
                wp = wpool.tile([P, NKP, 3, P], BF16, tag="wp")
                nc.sync.dma_start(wp, wp_d[:, j])

                # per-head layouts, uniform [content | pos] ordering:
                #   QT/KT [p=dk, head-in-pair, t]
                #   V     [p=token%128, tokenchunk, head, dv(cont|pos)]
                QT = qkt.tile([P, 2, L], BF16, tag="qt")
                KT = qkt.tile([P, 2, L], BF16, tag="kt")
                V = qkt.tile([P, 8, 2, P], BF16, tag="v")

                for s, DST in ((0, QT), (1, KT)):
                    for half in range(2):
                        hs = ds(half * 512, 512)
                        pc = ps_mm.tile([P, 512], F32, tag="mm")
                        for kc in range(NKC):
                            nc.tensor.matmul(
                                pc, wc[:, kc, s, :], XT[:, kc, hs],
                                start=(kc == 0), stop=(kc == NKC - 1))
                        pp = ps_mm.tile([P, 512], F32, tag="mm")
                        for kc in range(NKP):
                            nc.tensor.matmul(
                                pp, wp[:, kc, s, :], XT[:, NKC + kc, hs],
                                start=(kc == 0), stop=(kc == NKP - 1))
                        # full-partition psum->sbuf casts, then DMAs route
                        # the head halves (DMA can shift partitions)
                        sc = stg.tile([P, 512], BF16, tag="sc")
                        nc.any.tensor_copy(sc, pc)
                        sp = stg.tile([P, 512], BF16, tag="sp")
                        nc.any.tensor_copy(sp, pp)
                        nc.sync.dma_start(DST[lo, 0, hs], sc[lo])
                        nc.sync.dma_start(DST[lo, 1, hs], sc[hi])
                        nc.sync.dma_start(DST[hi, 0, hs], sp[lo])
                        nc.sync.dma_start(DST[hi, 1, hs], sp[hi])

                # V natural: out[token, dv] = sum_f X^T[f, token] * Wv[f, dv]
                for rc in range(8):
                    rsl = ds(rc * P, P)
                    pv_n = ps_mm.tile([P, 512], F32, tag="mm")
                    for kc in range(NKC):
                        nc.tensor.matmul(
                            pv_n[:, 0:128], XT[:, kc, rsl], wc[:, kc, 2, :],
                            start=(kc == 0), stop=(kc == NKC - 1))
                    for kc in range(NKP):
                        nc.tensor.matmul(
                            pv_n[:, 128:256], XT[:, NKC + kc, rsl],
                            wp[:, kc, 2, :],
                            start=(kc == 0), stop=(kc == NKP - 1))
                    # psum cols [h c | h' c | h p | h' p] -> per-head
                    # contiguous [cont|pos] blocks via a strided source AP
                    nc.any.tensor_copy(
                        V[:, rc],
                        pv_n[:, 0:256].rearrange(
                            "p (half head e) -> p head half e",
                            half=2, head=2))

                # ---- Phase B: attention for the two heads ---------------
                for hh in range(2):
                    vb = V[:, :, hh, :]   # [p, chunk, dv]
                    for half in range(2):
                        hs = ds(half * 512, 512)
                        pv = ps_pv.tile([P, 512], F32, tag="pv")
                        dd = ps_d.tile([P, 512], F32, tag="d")
                        for c in range(8):
                            pss = ps_mm.tile([P, 512], F32, tag="mm")
                            nc.tensor.matmul(
                                pss, KT[:, hh, ds(c * P, P)],
                                QT[:, hh, hs], start=True, stop=True)
                            e = epool.tile([P, 512], BF16, tag="e")
                            nc.scalar.activation(e, pss, AF.Exp,
                                                 scale=INV_TEMPER)
                            nc.tensor.matmul(
                                pv, vb[:, c], e,
                                start=(c == 0), stop=(c == 7))
                            nc.tensor.matmul(
                                dd, ones, e,
                                start=(c == 0), stop=(c == 7))

                        # normalize by 1/d (already replicated across psum
                        # partitions by the all-ones lhsT), stage, route
                        rd = dpool.tile([P, 512], F32, tag="rd")
                        nc.vector.reciprocal(rd, dd)   # psum -> sbuf
                        no = stg.tile([P, 512], BF16, tag="no")
                        nc.vector.tensor_mul(no, pv, rd)
                        # psum rows [o1 | o2] for every head; route to the
                        # packed operands
                        if hh == 0:
                            nc.sync.dma_start(O1T[lo, j, hs], no[lo])
                            nc.sync.dma_start(O2T[lo, j, hs], no[hi])
                        else:
                            nc.sync.dma_start(O1T[hi, j, hs], no[lo])
                            nc.sync.dma_start(O2T[hi, j, hs], no[hi])

            # ---- Phase C: output projection + residual + layernorm ------
            for t in range(L // P):
                tsl = ds(t * P, P)
                poa = ps_pv.tile([P, 512], F32, tag="pv")   # o1[:, 0:512]
                pob = ps_d.tile([P, 512], F32, tag="d")     # o1[:,512:768] | o2
                for kc in range(4):
                    nc.tensor.matmul(poa, O1T[:, kc, tsl],
                                     W1[:, kc, 0:512],
                                     start=kc == 0, stop=kc == 3)
                for kc in range(4):
                    nc.tensor.matmul(pob[:, 0:256], O1T[:, kc, tsl],
                                     W1[:, kc, 512:768],
                                     start=kc == 0, stop=kc == 3)
                for kc in range(4):
                    nc.tensor.matmul(pob[:, 256:512], O2T[:, kc, tsl],
                                     W2[:, kc, :],
                                     start=kc == 0, stop=kc == 3)

                xts = xpool.tile([P, DM], F32, tag="x")
                nc.sync.dma_start(xts, xr[tsl, :])
                z = zpool.tile([P, DM], F32, tag="z")
                nc.vector.tensor_add(z[:, 0:512], poa, xts[:, 0:512])
                nc.vector.tensor_add(z[:, 512:1024], pob, xts[:, 512:1024])

                stats = stat.tile([P, 2, 6], F32, tag="st")
                nc.vector.bn_stats(stats[:, 0], z[:, 0:512])
                nc.vector.bn_stats(stats[:, 1], z[:, 512:1024])
                mv = stat.tile([P, 2], F32, tag="mv")
                nc.vector.bn_aggr(mv, stats)
                sig = stat.tile([P, 1], F32, tag="sig")
                # unbiased std: sqrt(var * n/(n-1)), then +eps, then 1/x
                nc.scalar.activation(sig, mv[:, 1:2], AF.Sqrt,
                                     scale=float(DM) / (DM - 1))
                nc.vector.tensor_scalar_add(sig, sig, EPS)
                nc.vector.reciprocal(sig, sig)
                nc.vector.tensor_scalar(z, z, mv[:, 0:1], sig,
                                        ALU.subtract, ALU.mult)
                if apply_ln:
                    nc.vector.tensor_mul(z, z, LNA.to_broadcast((P, DM)))
                    nc.vector.tensor_add(z, z, LNB.to_broadcast((P, DM)))
                nc.sync.dma_start(out_d[tsl, :], z)

    nc.finalize()
    return nc


def _part_major(a, p=P):
    """[K*p, ...rest] -> [p, K, ...rest] contiguous (partition-major)."""
    k = a.shape[0] // p
    return np.ascontiguousarray(
        a.reshape((k, p) + a.shape[1:]).swapaxes(0, 1))


def _prep(inp, w_qs1, w_ks1, w_vs1, w_qs2, w_ks2, w_vs2, w_proj1, w_proj2):
    wc = np.empty((NPAIR, DC, 3, P), BF16NP)
    wp = np.empty((NPAIR, DP, 3, P), BF16NP)
    for j in range(NPAIR):
        for s, (wa, wb) in enumerate(((w_qs1, w_qs2), (w_ks1, w_ks2),
                                      (w_vs1, w_vs2))):
            wc[j, :, s, 0:64] = wa[2 * j]
            wc[j, :, s, 64:128] = wa[2 * j + 1]
            wp[j, :, s, 0:64] = wb[2 * j]
            wp[j, :, s, 64:128] = wb[2 * j + 1]
    # -> [P, NPAIR, NK, 3, P] partition-major
    wc = np.ascontiguousarray(
        wc.reshape(NPAIR, NKC, P, 3, P).transpose(2, 0, 1, 3, 4))
    wp = np.ascontiguousarray(
        wp.reshape(NPAIR, NKP, P, 3, P).transpose(2, 0, 1, 3, 4))
    w1 = _part_major(np.asarray(w_proj1, np.float32).astype(BF16NP))
    w2 = _part_major(np.asarray(w_proj2, np.float32).astype(BF16NP))

    x = np.ascontiguousarray(np.asarray(inp, np.float32)).reshape(NCORES, L, DM)
    xts = [_part_major(x[b].T.astype(BF16NP)) for b in range(NCORES)]
    return x, xts, wc, wp, w1, w2


_NC_CACHE = {}


def _get_nc(apply_ln):
    if apply_ln not in _NC_CACHE:
        _NC_CACHE[apply_ln] = build_nc(apply_ln)
    return _NC_CACHE[apply_ln]


def kernel(inp, w_qs1, w_ks1, w_vs1, w_qs2, w_ks2, w_vs2, w_proj1, w_proj2,
           ln_a, ln_b, batch_size, max_len, _trace=False):
    inp = np.asarray(inp, np.float32)
    assert int(batch_size) == NCORES and int(max_len) == L
    assert inp.shape == (NCORES * L, DM)

    ln_a = np.asarray(ln_a, np.float32).reshape(-1)
    ln_b = np.asarray(ln_b, np.float32).reshape(-1)
    apply_ln = not (np.all(ln_a == 1.0) and np.all(ln_b == 0.0))

    x, xts, wc, wp, w1, w2 = _prep(
        inp, np.asarray(w_qs1, np.float32), np.asarray(w_ks1, np.float32),
        np.asarray(w_vs1, np.float32), np.asarray(w_qs2, np.float32),
        np.asarray(w_ks2, np.float32), np.asarray(w_vs2, np.float32),
        np.asarray(w_proj1, np.float32), np.asarray(w_proj2, np.float32))

    nc = _get_nc(apply_ln)

    in_maps = []
    for b in range(NCORES):
        m = dict(xt=xts[b], xr=np.ascontiguousarray(x[b]),
                 wc=wc, wp=wp, w1=w1, w2=w2)
        if apply_ln:
            m["lna"] = ln_a.reshape(1, DM)
            m["lnb"] = ln_b.reshape(1, DM)
        in_maps.append(m)

    res = run_bass_kernel_spmd(nc, in_maps, list(range(NCORES)), trace=_trace)
    out = np.concatenate([res.results[b]["out"] for b in range(NCORES)], 0)
    if _trace:
        return out, res
    return out
